# revision 9
# baseline (speedup 1.0000x reference)
"""GCN encoder (3x GCNConv+ReLU+BN, global_add_pool) on 8 TRN2 NeuronCores.

Sharding: nodes dst-sharded contiguously (12500/core, padded 12544); per core
dsts are degree-sorted; each dst's in-edges (+self +bias edges) are dealt into
(chunk, round) gather columns. Per layer: y = dinv*(xa @ W^T) shard -> AllGather
-> per chunk: K=1 indirect-DMA row gathers, broadcast-TT edge-weight scale,
add-reduce over rounds, ACT relu with per-dst dinv -> x. BN affine applied
in-place for the next layer (stats via mask-matmul + AllReduce). Pooling =
host-built one-hot matmul per chunk. Host assembles/unpermutes outputs.
"""
import sys
sys.path.insert(0, "/opt/trn_rl_repo")
import numpy as np

N = 100000
E = 1600000
F = 128
DIM = 128
L = 3
G = 512
NC = 8
S = N // NC
SP = 12544
CH = SP // 128
BN_EPS = 1e-5
GW = 96
YROWS = NC * SP
BROW = S  # b-vector parked in core0's first pad slot


def _plan(edge_index, edge_weight, batch):
    row = edge_index[0].astype(np.int64)
    col = edge_index[1].astype(np.int64)
    ew = edge_weight.astype(np.float32)
    deg = np.bincount(col, weights=ew, minlength=N) + 1.0
    dinv = (1.0 / np.sqrt(deg)).astype(np.float32)

    cnt_in = np.bincount(col, minlength=N)
    perms = []
    node_slot = np.empty(N, np.int64)
    for k in range(NC):
        ids = np.arange(k * S, (k + 1) * S)
        order = np.argsort(-cnt_in[ids], kind="stable")
        perm = ids[order]
        perms.append(perm)
        node_slot[perm] = k * SP + np.arange(S)

    owner = col // S
    tot = cnt_in + 2
    Rc = np.zeros(CH, np.int64)
    for k in range(NC):
        t = tot[perms[k]]
        for c in range(CH):
            lo = c * 128
            if lo < S:
                Rc[c] = max(Rc[c], t[lo:min(lo + 128, S)].max())
    Rc = np.maximum(Rc, 1)
    Roff = np.concatenate([[0], np.cumsum(Rc)]).astype(np.int64)
    RT = int(Roff[-1])

    gidx = np.zeros((NC, 128, RT), np.int32)
    gew = np.zeros((NC, 128, RT), np.float32)
    dinv_sb = np.zeros((NC, 128, CH), np.float32)
    mask_sb = np.zeros((NC, 128, CH), np.float32)
    pool_p = np.zeros((NC, SP, GW), np.float32)
    g0s = []
    graph_of = batch.astype(np.int64)
    for k in range(NC):
        m = owner == k
        r_k, c_k, w_k = row[m], col[m], ew[m]
        slot_k = node_slot[c_k] - k * SP
        o = np.argsort(slot_k, kind="stable")
        r_k, w_k, slot_k = r_k[o], w_k[o], slot_k[o]
        counts = np.bincount(slot_k, minlength=SP)
        offs = np.concatenate([[0], np.cumsum(counts)])
        pos = np.arange(len(slot_k)) - offs[slot_k]
        colidx = Roff[slot_k // 128] + pos
        gidx[k, slot_k % 128, colidx] = node_slot[r_k].astype(np.int32)
        gew[k, slot_k % 128, colidx] = w_k
        sl = np.arange(S)
        perm = perms[k]
        self_col = Roff[sl // 128] + counts[sl]
        gidx[k, sl % 128, self_col] = (k * SP + sl).astype(np.int32)
        gew[k, sl % 128, self_col] = 1.0
        b_col = self_col + 1
        gidx[k, sl % 128, b_col] = BROW
        gew[k, sl % 128, b_col] = np.sqrt(deg[perm]).astype(np.float32)
        dv = np.zeros(SP, np.float32)
        dv[:S] = dinv[perm]
        dinv_sb[k] = dv.reshape(CH, 128).T
        mk = np.zeros(SP, np.float32)
        mk[:S] = 1.0
        mask_sb[k] = mk.reshape(CH, 128).T
        gids = graph_of[perm]
        g0 = int(gids.min())
        g0s.append(g0)
        gl = gids - g0
        assert gl.max() < GW, f"graph window overflow {gl.max()}"
        pool_p[k][np.arange(S), gl] = 1.0
    return dict(gidx=gidx, gew=gew, dinv_sb=dinv_sb, mask_sb=mask_sb,
                pool_p=pool_p, g0s=g0s, perms=perms,
                Rc=[int(v) for v in Rc], Roff=[int(v) for v in Roff], RT=RT)


def _build(Rc, Roff, RT):
    from concourse import bass, bacc, tile, mybir
    from concourse.masks import make_identity
    from contextlib import ExitStack
    DT = mybir.dt.float32
    IT = mybir.dt.int32
    AL = mybir.AluOpType
    ACTF = mybir.ActivationFunctionType

    nc = bacc.Bacc()
    x_in = nc.dram_tensor("x_in", [SP, F], DT, kind="ExternalInput")
    gidx_d = nc.dram_tensor("gidx", [128, RT], IT, kind="ExternalInput")
    gew_d = nc.dram_tensor("gew", [128, RT], DT, kind="ExternalInput")
    dinv_d = nc.dram_tensor("dinv_sb", [128, CH], DT, kind="ExternalInput")
    mask_d = nc.dram_tensor("mask_sb", [128, CH], DT, kind="ExternalInput")
    pool_d = nc.dram_tensor("pool_p", [SP, GW], DT, kind="ExternalInput")
    W_d = nc.dram_tensor("Wf", [L, DIM, DIM], DT, kind="ExternalInput")
    b_d = nc.dram_tensor("bv", [L, DIM], DT, kind="ExternalInput")
    gam_d = nc.dram_tensor("gam", [DIM, L], DT, kind="ExternalInput")
    bet_d = nc.dram_tensor("bet", [DIM, L], DT, kind="ExternalInput")

    xout = nc.dram_tensor("xout", [L, SP, F], DT, kind="ExternalOutput")
    stats_out = nc.dram_tensor("stats_out", [L, DIM, 2], DT, kind="ExternalOutput")
    pool_out = nc.dram_tensor("pool_out", [L, GW, DIM], DT, kind="ExternalOutput")

    with tile.TileContext(nc) as tc, \
         tc.tile_pool(name="dram", bufs=1, space="DRAM") as dram, \
         tc.tile_pool(name="sb", bufs=1) as sb, \
         tc.tile_pool(name="strp", bufs=2) as strp, \
         tc.tile_pool(name="accp", bufs=2) as accp, \
         tc.tile_pool(name="psA", bufs=2, space="PSUM") as psA, \
         tc.tile_pool(name="psB", bufs=1, space="PSUM") as psB:
        y_loc = dram.tile([SP, F], DT)
        y_alls = [dram.tile([YROWS, F], DT, addr_space="Shared", name=f"y_all{i}") for i in range(L)]
        st_loc = dram.tile([DIM, 2], DT)
        st_alls = [dram.tile([DIM, 2], DT, addr_space="Shared", name=f"st_all{i}") for i in range(L)]

        ident = sb.tile([128, 128], DT)
        make_identity(nc, ident[:])
        x_sb = sb.tile([128, CH, F], DT)
        xaT = sb.tile([128, SP], DT)
        gidx_sb = sb.tile([128, RT], IT)
        gew_sb = sb.tile([128, RT], DT)
        dinv_sb = sb.tile([128, CH], DT)
        mask_sb = sb.tile([128, CH], DT)
        wt_sb = sb.tile([128, DIM], DT)
        s_col = sb.tile([128, 1], DT)
        t_col = sb.tile([128, 1], DT)
        s_bc = sb.tile([128, 128], DT)
        t_bc = sb.tile([128, 128], DT)
        smallp = sb.tile([128, 8], DT)

        nc.sync.dma_start(gidx_sb[:], gidx_d[:])
        nc.sync.dma_start(gew_sb[:], gew_d[:])
        nc.sync.dma_start(dinv_sb[:], dinv_d[:])
        nc.sync.dma_start(mask_sb[:], mask_d[:])
        for c in range(CH):
            nc.sync.dma_start(x_sb[:, c, :], x_in[c * 128:(c + 1) * 128, :])

        for l in range(L):
            # xa^T (feat-major) from node-major x_sb
            for c in range(CH):
                pt = psA.tile([128, 128], DT, tag="tp")
                nc.tensor.transpose(out=pt[:], in_=x_sb[:, c, :], identity=ident[:])
                nc.vector.tensor_copy(out=xaT[:, c * 128:(c + 1) * 128], in_=pt[:])
            # W_l^T tile: W_d[l] is [out,in]; need [in,out]
            wl = strp.tile([128, DIM], DT, tag="wl")
            nc.sync.dma_start(wl[:], W_d[l])
            wpt = psA.tile([128, 128], DT, tag="tp")
            nc.tensor.transpose(out=wpt[:], in_=wl[:], identity=ident[:])
            nc.vector.tensor_copy(out=wt_sb[:], in_=wpt[:])
            # y shard
            for c in range(CH):
                yp = psA.tile([128, 128], DT, tag="yp")
                nc.tensor.matmul(out=yp[:], lhsT=xaT[:, c * 128:(c + 1) * 128],
                                 rhs=wt_sb[:], start=True, stop=True)
                ych = strp.tile([128, 128], DT, tag="ych")
                nc.vector.tensor_scalar_mul(ych[:], yp[:], dinv_sb[:, c:c + 1])
                nc.sync.dma_start(y_loc[c * 128:(c + 1) * 128, :], ych[:])
            brl = strp.tile([1, DIM], DT, tag="brl")
            nc.sync.dma_start(brl[:], b_d[l:l + 1, :])
            nc.sync.dma_start(y_loc[S:S + 1, :], brl[:])
            y_all = y_alls[l]
            nc.gpsimd.collective_compute(
                "AllGather", AL.bypass,
                replica_groups=[list(range(NC))],
                ins=[y_loc[:].opt()],
                outs=[y_all[:].opt()])
            # aggregation
            for c in range(CH):
                R = Rc[c]
                o0 = Roff[c]
                acc = accp.tile([128, max(Rc), 128], DT, tag="acc")
                for k in range(R):
                    nc.gpsimd.indirect_dma_start(
                        out=acc[:, k, :], out_offset=None,
                        in_=y_all[:],
                        in_offset=bass.IndirectOffsetOnAxis(
                            ap=gidx_sb[:, o0 + k:o0 + k + 1], axis=0))
                nc.vector.tensor_tensor(
                    out=acc[:, :R, :], in0=acc[:, :R, :],
                    in1=gew_sb[:, o0:o0 + R, None].to_broadcast([128, R, 128]),
                    op=AL.mult)
                red = strp.tile([128, 128], DT, tag="red")
                nc.vector.tensor_copy(out=red[:], in_=acc[:, 0, :])
                for k in range(1, R):
                    nc.vector.tensor_tensor(out=red[:], in0=red[:], in1=acc[:, k, :], op=AL.add)
                nc.scalar.activation(out=x_sb[:, c, :], in_=red[:],
                                     func=ACTF.Relu, scale=dinv_sb[:, c:c + 1])
                nc.sync.dma_start(xout[l][c * 128:(c + 1) * 128, :], x_sb[:, c, :])
            # stats
            s1p = psB.tile([128, 1], DT, tag="s1")
            s2p = psB.tile([128, 1], DT, tag="s2")
            for c in range(CH):
                xsq = strp.tile([128, 128], DT, tag="xsq")
                nc.vector.tensor_tensor(out=xsq[:], in0=x_sb[:, c, :], in1=x_sb[:, c, :], op=AL.mult)
                nc.tensor.matmul(out=s1p[:], lhsT=x_sb[:, c, :], rhs=mask_sb[:, c:c + 1],
                                 start=(c == 0), stop=(c == CH - 1))
                nc.tensor.matmul(out=s2p[:], lhsT=xsq[:], rhs=mask_sb[:, c:c + 1],
                                 start=(c == 0), stop=(c == CH - 1))
            stt = strp.tile([DIM, 2], DT, tag="stt")
            nc.vector.tensor_copy(out=stt[:, 0:1], in_=s1p[:])
            nc.vector.tensor_copy(out=stt[:, 1:2], in_=s2p[:])
            nc.sync.dma_start(st_loc[:], stt[:])
            st_all = st_alls[l]
            nc.gpsimd.collective_compute(
                "AllReduce", AL.add,
                replica_groups=[list(range(NC))],
                ins=[st_loc[:].opt()], outs=[st_all[:].opt()])
            sta = strp.tile([DIM, 2], DT, tag="sta")
            nc.sync.dma_start(sta[:], st_all[:])
            nc.sync.dma_start(stats_out[l], sta[:])
            # pooling of raw x
            pl = psB.tile([GW, 128], DT, tag="pl")
            for c in range(CH):
                plsb = strp.tile([128, GW], DT, tag="plsb")
                nc.sync.dma_start(plsb[:], pool_d[c * 128:(c + 1) * 128, :])
                nc.tensor.matmul(out=pl[:], lhsT=plsb[:], rhs=x_sb[:, c, :],
                                 start=(c == 0), stop=(c == CH - 1))
            plo = strp.tile([GW, 128], DT, tag="plo")
            nc.vector.tensor_copy(out=plo[:], in_=pl[:])
            nc.sync.dma_start(pool_out[l], plo[:])
            # BN affine for next layer, applied in-place to x_sb
            if l < L - 1:
                mu = smallp[:, 0:1]
                var = smallp[:, 1:2]
                inv = smallp[:, 2:3]
                musq = smallp[:, 3:4]
                sq = smallp[:, 4:5]
                ms = smallp[:, 5:6]
                nc.vector.tensor_scalar_mul(mu, sta[:, 0:1], 1.0 / float(N))
                nc.vector.tensor_scalar_mul(var, sta[:, 1:2], 1.0 / float(N))
                nc.vector.tensor_tensor(out=musq, in0=mu, in1=mu, op=AL.mult)
                nc.vector.tensor_tensor(out=var, in0=var, in1=musq, op=AL.subtract)
                nc.vector.tensor_scalar_add(var, var, BN_EPS)
                nc.scalar.activation(out=sq, in_=var, func=ACTF.Sqrt)
                nc.vector.reciprocal(out=inv, in_=sq)
                gcol = strp.tile([128, 1], DT, tag="gcol")
                bcol = strp.tile([128, 1], DT, tag="bcol")
                nc.sync.dma_start(gcol[:], gam_d[:, l:l + 1])
                nc.sync.dma_start(bcol[:], bet_d[:, l:l + 1])
                nc.vector.tensor_tensor(out=s_col[:], in0=gcol[:], in1=inv, op=AL.mult)
                nc.vector.tensor_tensor(out=ms, in0=mu, in1=s_col[:], op=AL.mult)
                nc.vector.tensor_tensor(out=t_col[:], in0=bcol[:], in1=ms, op=AL.subtract)
                sp2 = psA.tile([1, 128], DT, tag="tp")
                nc.tensor.transpose(out=sp2[:], in_=s_col[:], identity=ident[:])
                srow = strp.tile([1, 128], DT, tag="srow")
                nc.vector.tensor_copy(out=srow[:], in_=sp2[:])
                tp2 = psA.tile([1, 128], DT, tag="tp")
                nc.tensor.transpose(out=tp2[:], in_=t_col[:], identity=ident[:])
                trow = strp.tile([1, 128], DT, tag="trow")
                nc.vector.tensor_copy(out=trow[:], in_=tp2[:])
                nc.gpsimd.partition_broadcast(s_bc[:], srow[:])
                nc.gpsimd.partition_broadcast(t_bc[:], trow[:])
                for c in range(CH):
                    nc.vector.tensor_tensor(out=x_sb[:, c, :], in0=x_sb[:, c, :], in1=s_bc[:], op=AL.mult)
                    nc.vector.tensor_tensor(out=x_sb[:, c, :], in0=x_sb[:, c, :], in1=t_bc[:], op=AL.add)
    nc.compile()
    return nc


_CACHE = {}
LAST_EXEC_NS = None


def kernel(x, edge_index, edge_weight, batch, fc_w, W, b, gamma, beta):
    x = np.asarray(x, np.float32)
    W = np.asarray(W, np.float32)
    fc_w = np.asarray(fc_w, np.float32)
    b = np.asarray(b, np.float32)
    gamma = np.asarray(gamma, np.float32)
    beta = np.asarray(beta, np.float32)
    edge_index = np.asarray(edge_index)
    edge_weight = np.asarray(edge_weight, np.float32)
    batch = np.asarray(batch)

    plan = _plan(edge_index, edge_weight, batch)
    key = (tuple(plan["Rc"]), plan["RT"])
    if key not in _CACHE:
        _CACHE[key] = _build(plan["Rc"], plan["Roff"], plan["RT"])
    nc = _CACHE[key]

    Wf = np.stack([W[0] @ fc_w, W[1], W[2]]).astype(np.float32)
    in_maps = []
    for k in range(NC):
        perm = plan["perms"][k]
        xs = np.zeros((SP, F), np.float32)
        xs[:S] = x[perm]
        in_maps.append({
            "x_in": xs,
            "gidx": np.ascontiguousarray(plan["gidx"][k]),
            "gew": np.ascontiguousarray(plan["gew"][k]),
            "dinv_sb": np.ascontiguousarray(plan["dinv_sb"][k]),
            "mask_sb": np.ascontiguousarray(plan["mask_sb"][k]),
            "pool_p": np.ascontiguousarray(plan["pool_p"][k]),
            "Wf": Wf, "bv": b, "gam": np.ascontiguousarray(gamma.T), "bet": np.ascontiguousarray(beta.T),
        })

    from concourse.bass_utils import run_bass_kernel_spmd
    import os
    kw = {}
    if os.environ.get("GNN_TRACE"):
        try:
            import types as _t, antenv  # noqa
            from trn_agent_boot.trn_boot import _ntff_profile_via_ctypes
            _m = _t.ModuleType("antenv.axon_hooks")
            _m.get_axon_ntff_profile_hook = lambda: _ntff_profile_via_ctypes("/opt/axon/libaxon_pjrt.so")
            sys.modules.setdefault("antenv.axon_hooks", _m)
            kw = dict(trace=True, tmpdir=os.environ.get("GNN_TRACE_DIR") or None)
        except Exception:
            kw = {}
    res = run_bass_kernel_spmd(nc, in_maps, core_ids=list(range(NC)), **kw)
    global LAST_EXEC_NS
    LAST_EXEC_NS = res.exec_time_ns

    hcat = np.empty((N, L * DIM), np.float32)
    out = np.zeros((G, L * DIM), np.float32)
    st = res.results[0]["stats_out"]
    cnt = np.bincount(batch.astype(np.int64), minlength=G).astype(np.float32)
    for l in range(L):
        S1, S2 = st[l, :, 0], st[l, :, 1]
        mu = S1 / N
        var = S2 / N - mu * mu
        inv = 1.0 / np.sqrt(var + BN_EPS)
        s = gamma[l] * inv
        t = beta[l] - mu * s
        for k in range(NC):
            xr = res.results[k]["xout"][l][:S]
            hcat[plan["perms"][k], l * DIM:(l + 1) * DIM] = xr * s[None, :] + t[None, :]
            pr = res.results[k]["pool_out"][l]
            g0 = plan["g0s"][k]
            hi = min(G, g0 + GW)
            out[g0:hi, l * DIM:(l + 1) * DIM] += pr[:hi - g0] * s[None, :]
        out[:, l * DIM:(l + 1) * DIM] += cnt[:, None] * t[None, :]
    return out, hcat


# revision 10
# speedup vs baseline: 1.1225x; 1.1225x over previous
"""GCN encoder (3x GCNConv+ReLU+BN, global_add_pool) on 8 TRN2 NeuronCores.

Sharding: nodes dst-sharded contiguously (12500/core, padded 12544); per core
dsts are degree-sorted; each dst's in-edges (+self +bias edges) are dealt into
(chunk, round) gather columns. Per layer: y = dinv*(xa @ W^T) shard -> AllGather
-> per chunk: K=1 indirect-DMA row gathers, broadcast-TT edge-weight scale,
add-reduce over rounds, ACT relu with per-dst dinv -> x. BN affine applied
in-place for the next layer (stats via mask-matmul + AllReduce). Pooling =
host-built one-hot matmul per chunk. Host assembles/unpermutes outputs.
"""
import sys
sys.path.insert(0, "/opt/trn_rl_repo")
import numpy as np

N = 100000
E = 1600000
F = 128
DIM = 128
L = 3
G = 512
NC = 8
S = N // NC
SP = 12544
CH = SP // 128
BN_EPS = 1e-5
GW = 96
YROWS = NC * SP
BROW = S  # b-vector parked in core0's first pad slot


def _plan(edge_index, edge_weight, batch):
    row = edge_index[0].astype(np.int64)
    col = edge_index[1].astype(np.int64)
    ew = edge_weight.astype(np.float32)
    deg = np.bincount(col, weights=ew, minlength=N) + 1.0
    dinv = (1.0 / np.sqrt(deg)).astype(np.float32)

    cnt_in = np.bincount(col, minlength=N)
    perms = []
    node_slot = np.empty(N, np.int64)
    for k in range(NC):
        ids = np.arange(k * S, (k + 1) * S)
        order = np.argsort(-cnt_in[ids], kind="stable")
        perm = ids[order]
        perms.append(perm)
        node_slot[perm] = k * SP + np.arange(S)

    owner = col // S
    tot = np.maximum(cnt_in, 1)
    Rc = np.zeros(CH, np.int64)
    for k in range(NC):
        t = tot[perms[k]]
        for c in range(CH):
            lo = c * 128
            if lo < S:
                Rc[c] = max(Rc[c], t[lo:min(lo + 128, S)].max())
    Rc = np.maximum(Rc, 1)
    Roff = np.concatenate([[0], np.cumsum(Rc)]).astype(np.int64)
    RT = int(Roff[-1])

    gidx = np.zeros((NC, 128, RT), np.int32)
    gew = np.zeros((NC, 128, RT), np.float32)
    dinv_sb = np.zeros((NC, 128, CH), np.float32)
    sqd_sb = np.zeros((NC, 128, CH), np.float32)
    mask_sb = np.zeros((NC, 128, CH), np.float32)
    pool_p = np.zeros((NC, SP, GW), np.float32)
    g0s = []
    graph_of = batch.astype(np.int64)
    for k in range(NC):
        m = owner == k
        r_k, c_k, w_k = row[m], col[m], ew[m]
        slot_k = node_slot[c_k] - k * SP
        o = np.argsort(slot_k, kind="stable")
        r_k, w_k, slot_k = r_k[o], w_k[o], slot_k[o]
        counts = np.bincount(slot_k, minlength=SP)
        offs = np.concatenate([[0], np.cumsum(counts)])
        pos = np.arange(len(slot_k)) - offs[slot_k]
        colidx = Roff[slot_k // 128] + pos
        gidx[k, slot_k % 128, colidx] = node_slot[r_k].astype(np.int32)
        gew[k, slot_k % 128, colidx] = w_k
        perm = perms[k]
        sq = np.zeros(SP, np.float32)
        sq[:S] = np.sqrt(deg[perm]).astype(np.float32)
        sqd_sb[k] = sq.reshape(CH, 128).T
        dv = np.zeros(SP, np.float32)
        dv[:S] = dinv[perm]
        dinv_sb[k] = dv.reshape(CH, 128).T
        mk = np.zeros(SP, np.float32)
        mk[:S] = 1.0
        mask_sb[k] = mk.reshape(CH, 128).T
        gids = graph_of[perm]
        g0 = int(gids.min())
        g0s.append(g0)
        gl = gids - g0
        assert gl.max() < GW, f"graph window overflow {gl.max()}"
        pool_p[k][np.arange(S), gl] = 1.0
    return dict(gidx=gidx, gew=gew, dinv_sb=dinv_sb, sqd_sb=sqd_sb, mask_sb=mask_sb,
                pool_p=pool_p, g0s=g0s, perms=perms,
                Rc=[int(v) for v in Rc], Roff=[int(v) for v in Roff], RT=RT)


def _build(Rc, Roff, RT):
    from concourse import bass, bacc, tile, mybir
    from concourse.masks import make_identity
    from contextlib import ExitStack
    DT = mybir.dt.float32
    IT = mybir.dt.int32
    AL = mybir.AluOpType
    ACTF = mybir.ActivationFunctionType

    nc = bacc.Bacc()
    x_in = nc.dram_tensor("x_in", [SP, F], DT, kind="ExternalInput")
    gidx_d = nc.dram_tensor("gidx", [128, RT], IT, kind="ExternalInput")
    gew_d = nc.dram_tensor("gew", [128, RT], DT, kind="ExternalInput")
    dinv_d = nc.dram_tensor("dinv_sb", [128, CH], DT, kind="ExternalInput")
    sqd_d = nc.dram_tensor("sqd_sb", [128, CH], DT, kind="ExternalInput")
    mask_d = nc.dram_tensor("mask_sb", [128, CH], DT, kind="ExternalInput")
    pool_d = nc.dram_tensor("pool_p", [SP, GW], DT, kind="ExternalInput")
    W_d = nc.dram_tensor("Wf", [L, DIM, DIM], DT, kind="ExternalInput")
    b_d = nc.dram_tensor("bv", [L, DIM], DT, kind="ExternalInput")
    gam_d = nc.dram_tensor("gam", [DIM, L], DT, kind="ExternalInput")
    bet_d = nc.dram_tensor("bet", [DIM, L], DT, kind="ExternalInput")

    xout = nc.dram_tensor("xout", [L, SP, F], DT, kind="ExternalOutput")
    stats_out = nc.dram_tensor("stats_out", [L, DIM, 2], DT, kind="ExternalOutput")
    pool_out = nc.dram_tensor("pool_out", [L, GW, DIM], DT, kind="ExternalOutput")

    with tile.TileContext(nc) as tc, \
         tc.tile_pool(name="dram", bufs=1, space="DRAM") as dram, \
         tc.tile_pool(name="sb", bufs=1) as sb, \
         tc.tile_pool(name="strp", bufs=2) as strp, \
         tc.tile_pool(name="accp", bufs=2) as accp, \
         tc.tile_pool(name="psA", bufs=2, space="PSUM") as psA, \
         tc.tile_pool(name="psB", bufs=1, space="PSUM") as psB:
        y_loc = dram.tile([SP, F], DT)
        y_alls = [dram.tile([YROWS, F], DT, addr_space="Shared", name=f"y_all{i}") for i in range(L)]
        st_loc = dram.tile([DIM, 2], DT)
        st_alls = [dram.tile([DIM, 2], DT, addr_space="Shared", name=f"st_all{i}") for i in range(L)]

        ident = sb.tile([128, 128], DT)
        make_identity(nc, ident[:])
        x_sb = sb.tile([128, CH, F], DT)
        xaT = sb.tile([128, SP], DT)
        gidx_sb = sb.tile([128, RT], IT)
        gew_sb = sb.tile([128, RT], DT)
        dinv_sb = sb.tile([128, CH], DT)
        sqd_sb = sb.tile([128, CH], DT)
        mask_sb = sb.tile([128, CH], DT)
        b_bc = sb.tile([128, 128], DT)
        wt_sb = sb.tile([128, DIM], DT)
        s_col = sb.tile([128, 1], DT)
        t_col = sb.tile([128, 1], DT)
        s_bc = sb.tile([128, 128], DT)
        t_bc = sb.tile([128, 128], DT)
        smallp = sb.tile([128, 8], DT)

        nc.sync.dma_start(gidx_sb[:], gidx_d[:])
        nc.sync.dma_start(gew_sb[:], gew_d[:])
        nc.sync.dma_start(dinv_sb[:], dinv_d[:])
        nc.sync.dma_start(sqd_sb[:], sqd_d[:])
        nc.sync.dma_start(mask_sb[:], mask_d[:])
        for c in range(CH):
            nc.sync.dma_start(x_sb[:, c, :], x_in[c * 128:(c + 1) * 128, :])

        for l in range(L):
            # xa^T (feat-major) from node-major x_sb
            for c in range(CH):
                pt = psA.tile([128, 128], DT, tag="tp")
                nc.tensor.transpose(out=pt[:], in_=x_sb[:, c, :], identity=ident[:])
                nc.vector.tensor_copy(out=xaT[:, c * 128:(c + 1) * 128], in_=pt[:])
            # W_l^T tile: W_d[l] is [out,in]; need [in,out]
            wl = strp.tile([128, DIM], DT, tag="wl")
            nc.sync.dma_start(wl[:], W_d[l])
            wpt = psA.tile([128, 128], DT, tag="tp")
            nc.tensor.transpose(out=wpt[:], in_=wl[:], identity=ident[:])
            nc.vector.tensor_copy(out=wt_sb[:], in_=wpt[:])
            # y shard
            for c in range(CH):
                yp = psA.tile([128, 128], DT, tag="yp")
                nc.tensor.matmul(out=yp[:], lhsT=xaT[:, c * 128:(c + 1) * 128],
                                 rhs=wt_sb[:], start=True, stop=True)
                ych = strp.tile([128, 128], DT, tag="ych")
                nc.vector.tensor_scalar_mul(ych[:], yp[:], dinv_sb[:, c:c + 1])
                nc.sync.dma_start(y_loc[c * 128:(c + 1) * 128, :], ych[:])
            brl = strp.tile([1, DIM], DT, tag="brl")
            nc.sync.dma_start(brl[:], b_d[l:l + 1, :])
            nc.gpsimd.partition_broadcast(b_bc[:], brl[:])
            y_all = y_alls[l]
            nc.gpsimd.collective_compute(
                "AllGather", AL.bypass,
                replica_groups=[list(range(NC))],
                ins=[y_loc[:].opt()],
                outs=[y_all[:].opt()])
            # aggregation
            for c in range(CH):
                R = Rc[c]
                o0 = Roff[c]
                acc = accp.tile([128, max(Rc), 128], DT, tag="acc")
                for k in range(R):
                    nc.gpsimd.indirect_dma_start(
                        out=acc[:, k, :], out_offset=None,
                        in_=y_all[:],
                        in_offset=bass.IndirectOffsetOnAxis(
                            ap=gidx_sb[:, o0 + k:o0 + k + 1], axis=0))
                nc.vector.tensor_tensor(
                    out=acc[:, :R, :], in0=acc[:, :R, :],
                    in1=gew_sb[:, o0:o0 + R, None].to_broadcast([128, R, 128]),
                    op=AL.mult)
                # red = self term (own y rows, ew=1) + sqrt(deg)*b (bias edge)
                red = strp.tile([128, 128], DT, tag="red")
                nc.sync.dma_start(red[:], y_loc[c * 128:(c + 1) * 128, :])
                nc.vector.scalar_tensor_tensor(
                    out=red[:], in0=b_bc[:], scalar=sqd_sb[:, c:c + 1], in1=red[:],
                    op0=AL.mult, op1=AL.add)
                # pairwise-tree reduce of acc rounds
                h = R
                while h > 1:
                    half = h // 2
                    nc.vector.tensor_tensor(
                        out=acc[:, :half, :], in0=acc[:, :half, :],
                        in1=acc[:, h - half:h, :], op=AL.add)
                    h = h - half
                nc.vector.tensor_tensor(out=red[:], in0=red[:], in1=acc[:, 0, :], op=AL.add)
                nc.scalar.activation(out=x_sb[:, c, :], in_=red[:],
                                     func=ACTF.Relu, scale=dinv_sb[:, c:c + 1])
                nc.sync.dma_start(xout[l][c * 128:(c + 1) * 128, :], x_sb[:, c, :])
            # stats
            s1p = psB.tile([128, 1], DT, tag="s1")
            s2p = psB.tile([128, 1], DT, tag="s2")
            for c in range(CH):
                xsq = strp.tile([128, 128], DT, tag="xsq")
                nc.vector.tensor_tensor(out=xsq[:], in0=x_sb[:, c, :], in1=x_sb[:, c, :], op=AL.mult)
                nc.tensor.matmul(out=s1p[:], lhsT=x_sb[:, c, :], rhs=mask_sb[:, c:c + 1],
                                 start=(c == 0), stop=(c == CH - 1))
                nc.tensor.matmul(out=s2p[:], lhsT=xsq[:], rhs=mask_sb[:, c:c + 1],
                                 start=(c == 0), stop=(c == CH - 1))
            stt = strp.tile([DIM, 2], DT, tag="stt")
            nc.vector.tensor_copy(out=stt[:, 0:1], in_=s1p[:])
            nc.vector.tensor_copy(out=stt[:, 1:2], in_=s2p[:])
            nc.sync.dma_start(st_loc[:], stt[:])
            st_all = st_alls[l]
            nc.gpsimd.collective_compute(
                "AllReduce", AL.add,
                replica_groups=[list(range(NC))],
                ins=[st_loc[:].opt()], outs=[st_all[:].opt()])
            sta = strp.tile([DIM, 2], DT, tag="sta")
            nc.sync.dma_start(sta[:], st_all[:])
            nc.sync.dma_start(stats_out[l], sta[:])
            # pooling of raw x
            pl = psB.tile([GW, 128], DT, tag="pl")
            for c in range(CH):
                plsb = strp.tile([128, GW], DT, tag="plsb")
                nc.sync.dma_start(plsb[:], pool_d[c * 128:(c + 1) * 128, :])
                nc.tensor.matmul(out=pl[:], lhsT=plsb[:], rhs=x_sb[:, c, :],
                                 start=(c == 0), stop=(c == CH - 1))
            plo = strp.tile([GW, 128], DT, tag="plo")
            nc.vector.tensor_copy(out=plo[:], in_=pl[:])
            nc.sync.dma_start(pool_out[l], plo[:])
            # BN affine for next layer, applied in-place to x_sb
            if l < L - 1:
                mu = smallp[:, 0:1]
                var = smallp[:, 1:2]
                inv = smallp[:, 2:3]
                musq = smallp[:, 3:4]
                sq = smallp[:, 4:5]
                ms = smallp[:, 5:6]
                nc.vector.tensor_scalar_mul(mu, sta[:, 0:1], 1.0 / float(N))
                nc.vector.tensor_scalar_mul(var, sta[:, 1:2], 1.0 / float(N))
                nc.vector.tensor_tensor(out=musq, in0=mu, in1=mu, op=AL.mult)
                nc.vector.tensor_tensor(out=var, in0=var, in1=musq, op=AL.subtract)
                nc.vector.tensor_scalar_add(var, var, BN_EPS)
                nc.scalar.activation(out=sq, in_=var, func=ACTF.Sqrt)
                nc.vector.reciprocal(out=inv, in_=sq)
                gcol = strp.tile([128, 1], DT, tag="gcol")
                bcol = strp.tile([128, 1], DT, tag="bcol")
                nc.sync.dma_start(gcol[:], gam_d[:, l:l + 1])
                nc.sync.dma_start(bcol[:], bet_d[:, l:l + 1])
                nc.vector.tensor_tensor(out=s_col[:], in0=gcol[:], in1=inv, op=AL.mult)
                nc.vector.tensor_tensor(out=ms, in0=mu, in1=s_col[:], op=AL.mult)
                nc.vector.tensor_tensor(out=t_col[:], in0=bcol[:], in1=ms, op=AL.subtract)
                sp2 = psA.tile([1, 128], DT, tag="tp")
                nc.tensor.transpose(out=sp2[:], in_=s_col[:], identity=ident[:])
                srow = strp.tile([1, 128], DT, tag="srow")
                nc.vector.tensor_copy(out=srow[:], in_=sp2[:])
                tp2 = psA.tile([1, 128], DT, tag="tp")
                nc.tensor.transpose(out=tp2[:], in_=t_col[:], identity=ident[:])
                trow = strp.tile([1, 128], DT, tag="trow")
                nc.vector.tensor_copy(out=trow[:], in_=tp2[:])
                nc.gpsimd.partition_broadcast(s_bc[:], srow[:])
                nc.gpsimd.partition_broadcast(t_bc[:], trow[:])
                for c in range(CH):
                    nc.vector.tensor_tensor(out=x_sb[:, c, :], in0=x_sb[:, c, :], in1=s_bc[:], op=AL.mult)
                    nc.vector.tensor_tensor(out=x_sb[:, c, :], in0=x_sb[:, c, :], in1=t_bc[:], op=AL.add)
    nc.compile()
    return nc


_CACHE = {}
LAST_EXEC_NS = None


def kernel(x, edge_index, edge_weight, batch, fc_w, W, b, gamma, beta):
    x = np.asarray(x, np.float32)
    W = np.asarray(W, np.float32)
    fc_w = np.asarray(fc_w, np.float32)
    b = np.asarray(b, np.float32)
    gamma = np.asarray(gamma, np.float32)
    beta = np.asarray(beta, np.float32)
    edge_index = np.asarray(edge_index)
    edge_weight = np.asarray(edge_weight, np.float32)
    batch = np.asarray(batch)

    plan = _plan(edge_index, edge_weight, batch)
    key = (tuple(plan["Rc"]), plan["RT"])
    if key not in _CACHE:
        _CACHE[key] = _build(plan["Rc"], plan["Roff"], plan["RT"])
    nc = _CACHE[key]

    Wf = np.stack([W[0] @ fc_w, W[1], W[2]]).astype(np.float32)
    in_maps = []
    for k in range(NC):
        perm = plan["perms"][k]
        xs = np.zeros((SP, F), np.float32)
        xs[:S] = x[perm]
        in_maps.append({
            "x_in": xs,
            "gidx": np.ascontiguousarray(plan["gidx"][k]),
            "gew": np.ascontiguousarray(plan["gew"][k]),
            "dinv_sb": np.ascontiguousarray(plan["dinv_sb"][k]),
            "sqd_sb": np.ascontiguousarray(plan["sqd_sb"][k]),
            "mask_sb": np.ascontiguousarray(plan["mask_sb"][k]),
            "pool_p": np.ascontiguousarray(plan["pool_p"][k]),
            "Wf": Wf, "bv": b, "gam": np.ascontiguousarray(gamma.T), "bet": np.ascontiguousarray(beta.T),
        })

    from concourse.bass_utils import run_bass_kernel_spmd
    import os
    kw = {}
    if os.environ.get("GNN_TRACE"):
        try:
            import types as _t, antenv  # noqa
            from trn_agent_boot.trn_boot import _ntff_profile_via_ctypes
            _m = _t.ModuleType("antenv.axon_hooks")
            _m.get_axon_ntff_profile_hook = lambda: _ntff_profile_via_ctypes("/opt/axon/libaxon_pjrt.so")
            sys.modules.setdefault("antenv.axon_hooks", _m)
            kw = dict(trace=True, tmpdir=os.environ.get("GNN_TRACE_DIR") or None)
        except Exception:
            kw = {}
    res = run_bass_kernel_spmd(nc, in_maps, core_ids=list(range(NC)), **kw)
    global LAST_EXEC_NS
    LAST_EXEC_NS = res.exec_time_ns

    hcat = np.empty((N, L * DIM), np.float32)
    out = np.zeros((G, L * DIM), np.float32)
    st = res.results[0]["stats_out"]
    cnt = np.bincount(batch.astype(np.int64), minlength=G).astype(np.float32)
    for l in range(L):
        S1, S2 = st[l, :, 0], st[l, :, 1]
        mu = S1 / N
        var = S2 / N - mu * mu
        inv = 1.0 / np.sqrt(var + BN_EPS)
        s = gamma[l] * inv
        t = beta[l] - mu * s
        for k in range(NC):
            xr = res.results[k]["xout"][l][:S]
            hcat[plan["perms"][k], l * DIM:(l + 1) * DIM] = xr * s[None, :] + t[None, :]
            pr = res.results[k]["pool_out"][l]
            g0 = plan["g0s"][k]
            hi = min(G, g0 + GW)
            out[g0:hi, l * DIM:(l + 1) * DIM] += pr[:hi - g0] * s[None, :]
        out[:, l * DIM:(l + 1) * DIM] += cnt[:, None] * t[None, :]
    return out, hcat


# revision 11
# speedup vs baseline: 1.1815x; 1.0525x over previous
"""GCN encoder (3x GCNConv+ReLU+BN, global_add_pool) on 8 TRN2 NeuronCores.

Sharding: nodes dst-sharded contiguously (12500/core, padded 12544); per core
dsts are degree-sorted; each dst's in-edges (+self +bias edges) are dealt into
(chunk, round) gather columns. Per layer: y = dinv*(xa @ W^T) shard -> AllGather
-> per chunk: K=1 indirect-DMA row gathers, broadcast-TT edge-weight scale,
add-reduce over rounds, ACT relu with per-dst dinv -> x. BN affine applied
in-place for the next layer (stats via mask-matmul + AllReduce). Pooling =
host-built one-hot matmul per chunk. Host assembles/unpermutes outputs.
"""
import sys
sys.path.insert(0, "/opt/trn_rl_repo")
import numpy as np

N = 100000
E = 1600000
F = 128
DIM = 128
L = 3
G = 512
NC = 8
S = N // NC
SP = 12544
CH = SP // 128
BN_EPS = 1e-5
GW = 96
YROWS = NC * SP
BROW = S  # b-vector parked in core0's first pad slot


def _plan(edge_index, edge_weight, batch):
    row = edge_index[0].astype(np.int64)
    col = edge_index[1].astype(np.int64)
    ew = edge_weight.astype(np.float32)
    deg = np.bincount(col, weights=ew, minlength=N) + 1.0
    dinv = (1.0 / np.sqrt(deg)).astype(np.float32)

    cnt_in = np.bincount(col, minlength=N)
    perms = []
    node_slot = np.empty(N, np.int64)
    for k in range(NC):
        ids = np.arange(k * S, (k + 1) * S)
        order = np.argsort(-cnt_in[ids], kind="stable")
        perm = ids[order]
        perms.append(perm)
        node_slot[perm] = k * SP + np.arange(S)

    owner = col // S
    tot = np.maximum(cnt_in, 1)
    Rc = np.zeros(CH, np.int64)
    for k in range(NC):
        t = tot[perms[k]]
        for c in range(CH):
            lo = c * 128
            if lo < S:
                Rc[c] = max(Rc[c], t[lo:min(lo + 128, S)].max())
    Rc = np.maximum(Rc, 1)
    Roff = np.concatenate([[0], np.cumsum(Rc)]).astype(np.int64)
    RT = int(Roff[-1])

    gidx = np.zeros((NC, 128, RT), np.int32)
    gew = np.zeros((NC, 128, RT), np.float32)
    dinv_sb = np.zeros((NC, 128, CH), np.float32)
    sqd_sb = np.zeros((NC, 128, CH), np.float32)
    mask_sb = np.zeros((NC, 128, CH), np.float32)
    pool_p = np.zeros((NC, SP, GW), np.float32)
    g0s = []
    graph_of = batch.astype(np.int64)
    for k in range(NC):
        m = owner == k
        r_k, c_k, w_k = row[m], col[m], ew[m]
        slot_k = node_slot[c_k] - k * SP
        o = np.argsort(slot_k, kind="stable")
        r_k, w_k, slot_k = r_k[o], w_k[o], slot_k[o]
        counts = np.bincount(slot_k, minlength=SP)
        offs = np.concatenate([[0], np.cumsum(counts)])
        pos = np.arange(len(slot_k)) - offs[slot_k]
        colidx = Roff[slot_k // 128] + pos
        gidx[k, slot_k % 128, colidx] = node_slot[r_k].astype(np.int32)
        gew[k, slot_k % 128, colidx] = w_k
        perm = perms[k]
        sq = np.zeros(SP, np.float32)
        sq[:S] = np.sqrt(deg[perm]).astype(np.float32)
        sqd_sb[k] = sq.reshape(CH, 128).T
        dv = np.zeros(SP, np.float32)
        dv[:S] = dinv[perm]
        dinv_sb[k] = dv.reshape(CH, 128).T
        mk = np.zeros(SP, np.float32)
        mk[:S] = 1.0
        mask_sb[k] = mk.reshape(CH, 128).T
        gids = graph_of[perm]
        g0 = int(gids.min())
        g0s.append(g0)
        gl = gids - g0
        assert gl.max() < GW, f"graph window overflow {gl.max()}"
        pool_p[k][np.arange(S), gl] = 1.0
    return dict(gidx=gidx, gew=gew, dinv_sb=dinv_sb, sqd_sb=sqd_sb, mask_sb=mask_sb,
                pool_p=pool_p, g0s=g0s, perms=perms,
                Rc=[int(v) for v in Rc], Roff=[int(v) for v in Roff], RT=RT)


def _build(Rc, Roff, RT):
    from concourse import bass, bacc, tile, mybir
    from concourse.masks import make_identity
    from contextlib import ExitStack
    DT = mybir.dt.float32
    IT = mybir.dt.int32
    AL = mybir.AluOpType
    ACTF = mybir.ActivationFunctionType

    nc = bacc.Bacc()
    x_in = nc.dram_tensor("x_in", [SP, F], DT, kind="ExternalInput")
    gidx_d = nc.dram_tensor("gidx", [128, RT], IT, kind="ExternalInput")
    gew_d = nc.dram_tensor("gew", [128, RT], DT, kind="ExternalInput")
    dinv_d = nc.dram_tensor("dinv_sb", [128, CH], DT, kind="ExternalInput")
    sqd_d = nc.dram_tensor("sqd_sb", [128, CH], DT, kind="ExternalInput")
    mask_d = nc.dram_tensor("mask_sb", [128, CH], DT, kind="ExternalInput")
    pool_d = nc.dram_tensor("pool_p", [SP, GW], DT, kind="ExternalInput")
    W_d = nc.dram_tensor("Wf", [L, DIM, DIM], DT, kind="ExternalInput")
    b_d = nc.dram_tensor("bv", [L, DIM], DT, kind="ExternalInput")
    gam_d = nc.dram_tensor("gam", [DIM, L], DT, kind="ExternalInput")
    bet_d = nc.dram_tensor("bet", [DIM, L], DT, kind="ExternalInput")

    xout = nc.dram_tensor("xout", [L, SP, F], DT, kind="ExternalOutput")
    stats_out = nc.dram_tensor("stats_out", [L, DIM, 2], DT, kind="ExternalOutput")
    pool_out = nc.dram_tensor("pool_out", [L, GW, DIM], DT, kind="ExternalOutput")

    with tile.TileContext(nc) as tc, \
         tc.tile_pool(name="dram", bufs=1, space="DRAM") as dram, \
         tc.tile_pool(name="sb", bufs=1) as sb, \
         tc.tile_pool(name="strp", bufs=2) as strp, \
         tc.tile_pool(name="accp", bufs=2) as accp, \
         tc.tile_pool(name="psA", bufs=2, space="PSUM") as psA, \
         tc.tile_pool(name="psB", bufs=1, space="PSUM") as psB:
        y_loc = dram.tile([SP, F], DT)
        y_alls = [dram.tile([YROWS, F], DT, addr_space="Shared", name=f"y_all{i}") for i in range(L)]
        st_loc = dram.tile([DIM, 2], DT)
        st_alls = [dram.tile([DIM, 2], DT, addr_space="Shared", name=f"st_all{i}") for i in range(L)]

        ident = sb.tile([128, 128], DT)
        make_identity(nc, ident[:])
        x_sb = sb.tile([128, CH, F], DT)
        xaT = sb.tile([128, SP], DT)
        gidx_sb = sb.tile([128, RT], IT)
        gew_sb = sb.tile([128, RT], DT)
        dinv_sb = sb.tile([128, CH], DT)
        sqd_sb = sb.tile([128, CH], DT)
        mask_sb = sb.tile([128, CH], DT)
        b_bc = sb.tile([128, 128], DT)
        wt_sb = sb.tile([128, DIM], DT)
        s_col = sb.tile([128, 1], DT)
        t_col = sb.tile([128, 1], DT)
        s_bc = sb.tile([128, 128], DT)
        t_bc = sb.tile([128, 128], DT)
        smallp = sb.tile([128, 8], DT)

        nc.sync.dma_start(gidx_sb[:], gidx_d[:])
        nc.sync.dma_start(gew_sb[:], gew_d[:])
        nc.sync.dma_start(dinv_sb[:], dinv_d[:])
        nc.sync.dma_start(sqd_sb[:], sqd_d[:])
        nc.sync.dma_start(mask_sb[:], mask_d[:])
        for c in range(CH):
            nc.sync.dma_start(x_sb[:, c, :], x_in[c * 128:(c + 1) * 128, :])

        for l in range(L):
            # xa^T (feat-major) from node-major x_sb
            for c in range(CH):
                pt = psA.tile([128, 128], DT, tag="tp")
                nc.tensor.transpose(out=pt[:], in_=x_sb[:, c, :], identity=ident[:])
                nc.vector.tensor_copy(out=xaT[:, c * 128:(c + 1) * 128], in_=pt[:])
            # W_l^T tile: W_d[l] is [out,in]; need [in,out]
            wl = strp.tile([128, DIM], DT, tag="wl")
            nc.sync.dma_start(wl[:], W_d[l])
            wpt = psA.tile([128, 128], DT, tag="tp")
            nc.tensor.transpose(out=wpt[:], in_=wl[:], identity=ident[:])
            nc.vector.tensor_copy(out=wt_sb[:], in_=wpt[:])
            # y shard
            for c in range(CH):
                yp = psA.tile([128, 128], DT, tag="yp")
                nc.tensor.matmul(out=yp[:], lhsT=xaT[:, c * 128:(c + 1) * 128],
                                 rhs=wt_sb[:], start=True, stop=True)
                ych = strp.tile([128, 128], DT, tag="ych")
                nc.vector.tensor_scalar_mul(ych[:], yp[:], dinv_sb[:, c:c + 1])
                nc.sync.dma_start(y_loc[c * 128:(c + 1) * 128, :], ych[:])
            brl = strp.tile([1, DIM], DT, tag="brl")
            nc.sync.dma_start(brl[:], b_d[l:l + 1, :])
            nc.gpsimd.partition_broadcast(b_bc[:], brl[:])
            y_all = y_alls[l]
            nc.gpsimd.collective_compute(
                "AllGather", AL.bypass,
                replica_groups=[list(range(NC))],
                ins=[y_loc[:].opt()],
                outs=[y_all[:].opt()])
            # aggregation
            for c in range(CH):
                R = Rc[c]
                o0 = Roff[c]
                acc = accp.tile([128, max(Rc), 128], DT, tag="acc")
                for k in range(R):
                    nc.gpsimd.indirect_dma_start(
                        out=acc[:, k, :], out_offset=None,
                        in_=y_all[:],
                        in_offset=bass.IndirectOffsetOnAxis(
                            ap=gidx_sb[:, o0 + k:o0 + k + 1], axis=0))
                # red = self term (own y rows, ew=1) + sqrt(deg)*b (bias edge)
                red = strp.tile([128, 128], DT, tag="red")
                nc.sync.dma_start(red[:], y_loc[c * 128:(c + 1) * 128, :])
                nc.vector.scalar_tensor_tensor(
                    out=red[:], in0=b_bc[:], scalar=sqd_sb[:, c:c + 1], in1=red[:],
                    op0=AL.mult, op1=AL.add)
                # fused per-round scale+accumulate: red = acc_k * ew_k + red
                for k in range(R):
                    nc.vector.scalar_tensor_tensor(
                        out=red[:], in0=acc[:, k, :], scalar=gew_sb[:, o0 + k:o0 + k + 1],
                        in1=red[:], op0=AL.mult, op1=AL.add)
                nc.scalar.activation(out=x_sb[:, c, :], in_=red[:],
                                     func=ACTF.Relu, scale=dinv_sb[:, c:c + 1])
                nc.sync.dma_start(xout[l][c * 128:(c + 1) * 128, :], x_sb[:, c, :])
            # stats
            s1p = psB.tile([128, 1], DT, tag="s1")
            s2p = psB.tile([128, 1], DT, tag="s2")
            for c in range(CH):
                xsq = strp.tile([128, 128], DT, tag="xsq")
                nc.vector.tensor_tensor(out=xsq[:], in0=x_sb[:, c, :], in1=x_sb[:, c, :], op=AL.mult)
                nc.tensor.matmul(out=s1p[:], lhsT=x_sb[:, c, :], rhs=mask_sb[:, c:c + 1],
                                 start=(c == 0), stop=(c == CH - 1))
                nc.tensor.matmul(out=s2p[:], lhsT=xsq[:], rhs=mask_sb[:, c:c + 1],
                                 start=(c == 0), stop=(c == CH - 1))
            stt = strp.tile([DIM, 2], DT, tag="stt")
            nc.vector.tensor_copy(out=stt[:, 0:1], in_=s1p[:])
            nc.vector.tensor_copy(out=stt[:, 1:2], in_=s2p[:])
            nc.sync.dma_start(st_loc[:], stt[:])
            st_all = st_alls[l]
            nc.gpsimd.collective_compute(
                "AllReduce", AL.add,
                replica_groups=[list(range(NC))],
                ins=[st_loc[:].opt()], outs=[st_all[:].opt()])
            sta = strp.tile([DIM, 2], DT, tag="sta")
            nc.sync.dma_start(sta[:], st_all[:])
            nc.sync.dma_start(stats_out[l], sta[:])
            # pooling of raw x
            pl = psB.tile([GW, 128], DT, tag="pl")
            for c in range(CH):
                plsb = strp.tile([128, GW], DT, tag="plsb")
                nc.sync.dma_start(plsb[:], pool_d[c * 128:(c + 1) * 128, :])
                nc.tensor.matmul(out=pl[:], lhsT=plsb[:], rhs=x_sb[:, c, :],
                                 start=(c == 0), stop=(c == CH - 1))
            plo = strp.tile([GW, 128], DT, tag="plo")
            nc.vector.tensor_copy(out=plo[:], in_=pl[:])
            nc.sync.dma_start(pool_out[l], plo[:])
            # BN affine for next layer, applied in-place to x_sb
            if l < L - 1:
                mu = smallp[:, 0:1]
                var = smallp[:, 1:2]
                inv = smallp[:, 2:3]
                musq = smallp[:, 3:4]
                sq = smallp[:, 4:5]
                ms = smallp[:, 5:6]
                nc.vector.tensor_scalar_mul(mu, sta[:, 0:1], 1.0 / float(N))
                nc.vector.tensor_scalar_mul(var, sta[:, 1:2], 1.0 / float(N))
                nc.vector.tensor_tensor(out=musq, in0=mu, in1=mu, op=AL.mult)
                nc.vector.tensor_tensor(out=var, in0=var, in1=musq, op=AL.subtract)
                nc.vector.tensor_scalar_add(var, var, BN_EPS)
                nc.scalar.activation(out=sq, in_=var, func=ACTF.Sqrt)
                nc.vector.reciprocal(out=inv, in_=sq)
                gcol = strp.tile([128, 1], DT, tag="gcol")
                bcol = strp.tile([128, 1], DT, tag="bcol")
                nc.sync.dma_start(gcol[:], gam_d[:, l:l + 1])
                nc.sync.dma_start(bcol[:], bet_d[:, l:l + 1])
                nc.vector.tensor_tensor(out=s_col[:], in0=gcol[:], in1=inv, op=AL.mult)
                nc.vector.tensor_tensor(out=ms, in0=mu, in1=s_col[:], op=AL.mult)
                nc.vector.tensor_tensor(out=t_col[:], in0=bcol[:], in1=ms, op=AL.subtract)
                sp2 = psA.tile([1, 128], DT, tag="tp")
                nc.tensor.transpose(out=sp2[:], in_=s_col[:], identity=ident[:])
                srow = strp.tile([1, 128], DT, tag="srow")
                nc.vector.tensor_copy(out=srow[:], in_=sp2[:])
                tp2 = psA.tile([1, 128], DT, tag="tp")
                nc.tensor.transpose(out=tp2[:], in_=t_col[:], identity=ident[:])
                trow = strp.tile([1, 128], DT, tag="trow")
                nc.vector.tensor_copy(out=trow[:], in_=tp2[:])
                nc.gpsimd.partition_broadcast(s_bc[:], srow[:])
                nc.gpsimd.partition_broadcast(t_bc[:], trow[:])
                for c in range(CH):
                    nc.vector.tensor_tensor(out=x_sb[:, c, :], in0=x_sb[:, c, :], in1=s_bc[:], op=AL.mult)
                    nc.vector.tensor_tensor(out=x_sb[:, c, :], in0=x_sb[:, c, :], in1=t_bc[:], op=AL.add)
    nc.compile()
    return nc


_CACHE = {}
LAST_EXEC_NS = None


def kernel(x, edge_index, edge_weight, batch, fc_w, W, b, gamma, beta):
    x = np.asarray(x, np.float32)
    W = np.asarray(W, np.float32)
    fc_w = np.asarray(fc_w, np.float32)
    b = np.asarray(b, np.float32)
    gamma = np.asarray(gamma, np.float32)
    beta = np.asarray(beta, np.float32)
    edge_index = np.asarray(edge_index)
    edge_weight = np.asarray(edge_weight, np.float32)
    batch = np.asarray(batch)

    plan = _plan(edge_index, edge_weight, batch)
    key = (tuple(plan["Rc"]), plan["RT"])
    if key not in _CACHE:
        _CACHE[key] = _build(plan["Rc"], plan["Roff"], plan["RT"])
    nc = _CACHE[key]

    Wf = np.stack([W[0] @ fc_w, W[1], W[2]]).astype(np.float32)
    in_maps = []
    for k in range(NC):
        perm = plan["perms"][k]
        xs = np.zeros((SP, F), np.float32)
        xs[:S] = x[perm]
        in_maps.append({
            "x_in": xs,
            "gidx": np.ascontiguousarray(plan["gidx"][k]),
            "gew": np.ascontiguousarray(plan["gew"][k]),
            "dinv_sb": np.ascontiguousarray(plan["dinv_sb"][k]),
            "sqd_sb": np.ascontiguousarray(plan["sqd_sb"][k]),
            "mask_sb": np.ascontiguousarray(plan["mask_sb"][k]),
            "pool_p": np.ascontiguousarray(plan["pool_p"][k]),
            "Wf": Wf, "bv": b, "gam": np.ascontiguousarray(gamma.T), "bet": np.ascontiguousarray(beta.T),
        })

    from concourse.bass_utils import run_bass_kernel_spmd
    import os
    kw = {}
    if os.environ.get("GNN_TRACE"):
        try:
            import types as _t, antenv  # noqa
            from trn_agent_boot.trn_boot import _ntff_profile_via_ctypes
            _m = _t.ModuleType("antenv.axon_hooks")
            _m.get_axon_ntff_profile_hook = lambda: _ntff_profile_via_ctypes("/opt/axon/libaxon_pjrt.so")
            sys.modules.setdefault("antenv.axon_hooks", _m)
            kw = dict(trace=True, tmpdir=os.environ.get("GNN_TRACE_DIR") or None)
        except Exception:
            kw = {}
    res = run_bass_kernel_spmd(nc, in_maps, core_ids=list(range(NC)), **kw)
    global LAST_EXEC_NS
    LAST_EXEC_NS = res.exec_time_ns

    hcat = np.empty((N, L * DIM), np.float32)
    out = np.zeros((G, L * DIM), np.float32)
    st = res.results[0]["stats_out"]
    cnt = np.bincount(batch.astype(np.int64), minlength=G).astype(np.float32)
    for l in range(L):
        S1, S2 = st[l, :, 0], st[l, :, 1]
        mu = S1 / N
        var = S2 / N - mu * mu
        inv = 1.0 / np.sqrt(var + BN_EPS)
        s = gamma[l] * inv
        t = beta[l] - mu * s
        for k in range(NC):
            xr = res.results[k]["xout"][l][:S]
            hcat[plan["perms"][k], l * DIM:(l + 1) * DIM] = xr * s[None, :] + t[None, :]
            pr = res.results[k]["pool_out"][l]
            g0 = plan["g0s"][k]
            hi = min(G, g0 + GW)
            out[g0:hi, l * DIM:(l + 1) * DIM] += pr[:hi - g0] * s[None, :]
        out[:, l * DIM:(l + 1) * DIM] += cnt[:, None] * t[None, :]
    return out, hcat


# revision 12
# speedup vs baseline: 1.1987x; 1.0146x over previous
"""GCN encoder (3x GCNConv+ReLU+BN, global_add_pool) on 8 TRN2 NeuronCores.

Sharding: nodes dst-sharded contiguously (12500/core, padded 12544); per core
dsts are degree-sorted; each dst's in-edges (+self +bias edges) are dealt into
(chunk, round) gather columns. Per layer: y = dinv*(xa @ W^T) shard -> AllGather
-> per chunk: K=1 indirect-DMA row gathers, broadcast-TT edge-weight scale,
add-reduce over rounds, ACT relu with per-dst dinv -> x. BN affine applied
in-place for the next layer (stats via mask-matmul + AllReduce). Pooling =
host-built one-hot matmul per chunk. Host assembles/unpermutes outputs.
"""
import sys
sys.path.insert(0, "/opt/trn_rl_repo")
import numpy as np

N = 100000
E = 1600000
F = 128
DIM = 128
L = 3
G = 512
NC = 8
S = N // NC
SP = 12544
CH = SP // 128
BN_EPS = 1e-5
GW = 96
YROWS = NC * SP
BROW = S  # b-vector parked in core0's first pad slot


def _plan(edge_index, edge_weight, batch):
    row = edge_index[0].astype(np.int64)
    col = edge_index[1].astype(np.int64)
    ew = edge_weight.astype(np.float32)
    deg = np.bincount(col, weights=ew, minlength=N) + 1.0
    dinv = (1.0 / np.sqrt(deg)).astype(np.float32)

    cnt_in = np.bincount(col, minlength=N)
    perms = []
    node_slot = np.empty(N, np.int64)
    for k in range(NC):
        ids = np.arange(k * S, (k + 1) * S)
        order = np.argsort(-cnt_in[ids], kind="stable")
        perm = ids[order]
        perms.append(perm)
        node_slot[perm] = k * SP + np.arange(S)

    owner = col // S
    tot = np.maximum(cnt_in, 1)
    Rc = np.zeros(CH, np.int64)
    for k in range(NC):
        t = tot[perms[k]]
        for c in range(CH):
            lo = c * 128
            if lo < S:
                Rc[c] = max(Rc[c], t[lo:min(lo + 128, S)].max())
    Rc = np.maximum(Rc, 1)
    Roff = np.concatenate([[0], np.cumsum(Rc)]).astype(np.int64)
    RT = int(Roff[-1])

    gidx = np.zeros((NC, 128, RT), np.int32)
    gew = np.zeros((NC, 128, RT), np.float32)
    dinv_sb = np.zeros((NC, 128, CH), np.float32)
    sqd_sb = np.zeros((NC, 128, CH), np.float32)
    mask_sb = np.zeros((NC, 128, CH), np.float32)
    pool_p = np.zeros((NC, SP, GW), np.float32)
    g0s = []
    graph_of = batch.astype(np.int64)
    for k in range(NC):
        m = owner == k
        r_k, c_k, w_k = row[m], col[m], ew[m]
        slot_k = node_slot[c_k] - k * SP
        o = np.argsort(slot_k, kind="stable")
        r_k, w_k, slot_k = r_k[o], w_k[o], slot_k[o]
        counts = np.bincount(slot_k, minlength=SP)
        offs = np.concatenate([[0], np.cumsum(counts)])
        pos = np.arange(len(slot_k)) - offs[slot_k]
        colidx = Roff[slot_k // 128] + pos
        gidx[k, slot_k % 128, colidx] = node_slot[r_k].astype(np.int32)
        gew[k, slot_k % 128, colidx] = w_k
        perm = perms[k]
        sq = np.zeros(SP, np.float32)
        sq[:S] = np.sqrt(deg[perm]).astype(np.float32)
        sqd_sb[k] = sq.reshape(CH, 128).T
        dv = np.zeros(SP, np.float32)
        dv[:S] = dinv[perm]
        dinv_sb[k] = dv.reshape(CH, 128).T
        mk = np.zeros(SP, np.float32)
        mk[:S] = 1.0
        mask_sb[k] = mk.reshape(CH, 128).T
        gids = graph_of[perm]
        g0 = int(gids.min())
        g0s.append(g0)
        gl = gids - g0
        assert gl.max() < GW, f"graph window overflow {gl.max()}"
        pool_p[k][np.arange(S), gl] = 1.0
    return dict(gidx=gidx, gew=gew, dinv_sb=dinv_sb, sqd_sb=sqd_sb, mask_sb=mask_sb,
                pool_p=pool_p, g0s=g0s, perms=perms,
                Rc=[int(v) for v in Rc], Roff=[int(v) for v in Roff], RT=RT)


def _build(Rc, Roff, RT):
    from concourse import bass, bacc, tile, mybir
    from concourse.masks import make_identity
    from contextlib import ExitStack
    DT = mybir.dt.float32
    IT = mybir.dt.int32
    AL = mybir.AluOpType
    ACTF = mybir.ActivationFunctionType

    nc = bacc.Bacc()
    x_in = nc.dram_tensor("x_in", [SP, F], DT, kind="ExternalInput")
    gidx_d = nc.dram_tensor("gidx", [128, RT], IT, kind="ExternalInput")
    gew_d = nc.dram_tensor("gew", [128, RT], DT, kind="ExternalInput")
    dinv_d = nc.dram_tensor("dinv_sb", [128, CH], DT, kind="ExternalInput")
    sqd_d = nc.dram_tensor("sqd_sb", [128, CH], DT, kind="ExternalInput")
    mask_d = nc.dram_tensor("mask_sb", [128, CH], DT, kind="ExternalInput")
    pool_d = nc.dram_tensor("pool_p", [SP, GW], DT, kind="ExternalInput")
    W_d = nc.dram_tensor("Wf", [L, DIM, DIM], DT, kind="ExternalInput")
    b_d = nc.dram_tensor("bv", [L, DIM], DT, kind="ExternalInput")
    gam_d = nc.dram_tensor("gam", [DIM, L], DT, kind="ExternalInput")
    bet_d = nc.dram_tensor("bet", [DIM, L], DT, kind="ExternalInput")

    xout = nc.dram_tensor("xout", [L, SP, F], DT, kind="ExternalOutput")
    stats_out = nc.dram_tensor("stats_out", [L, DIM, 2], DT, kind="ExternalOutput")
    pool_out = nc.dram_tensor("pool_out", [L, GW, DIM], DT, kind="ExternalOutput")

    with tile.TileContext(nc) as tc, \
         tc.tile_pool(name="dram", bufs=1, space="DRAM") as dram, \
         tc.tile_pool(name="sb", bufs=1) as sb, \
         tc.tile_pool(name="strp", bufs=2) as strp, \
         tc.tile_pool(name="accp", bufs=2) as accp, \
         tc.tile_pool(name="psA", bufs=2, space="PSUM") as psA, \
         tc.tile_pool(name="psB", bufs=1, space="PSUM") as psB:
        y_loc = dram.tile([SP, F], DT)
        y_alls = [dram.tile([YROWS, F], DT, addr_space="Shared", name=f"y_all{i}") for i in range(L)]
        st_loc = dram.tile([DIM, 2], DT)
        st_alls = [dram.tile([DIM, 2], DT, addr_space="Shared", name=f"st_all{i}") for i in range(L)]

        ident = sb.tile([128, 128], DT)
        make_identity(nc, ident[:])
        x_sb = sb.tile([128, CH, F], DT)
        xaT = sb.tile([128, SP], DT)
        gidx_sb = sb.tile([128, RT], IT)
        gew_sb = sb.tile([128, RT], DT)
        dinv_sb = sb.tile([128, CH], DT)
        sqd_sb = sb.tile([128, CH], DT)
        mask_sb = sb.tile([128, CH], DT)
        b_bc = sb.tile([128, 128], DT)
        wt_sb = sb.tile([128, DIM], DT)
        s_col = sb.tile([128, 1], DT)
        t_col = sb.tile([128, 1], DT)
        s_bc = sb.tile([128, 128], DT)
        t_bc = sb.tile([128, 128], DT)
        smallp = sb.tile([128, 8], DT)

        nc.sync.dma_start(gidx_sb[:], gidx_d[:])
        nc.sync.dma_start(gew_sb[:], gew_d[:])
        nc.sync.dma_start(dinv_sb[:], dinv_d[:])
        nc.sync.dma_start(sqd_sb[:], sqd_d[:])
        nc.sync.dma_start(mask_sb[:], mask_d[:])
        for c in range(CH):
            nc.sync.dma_start(x_sb[:, c, :], x_in[c * 128:(c + 1) * 128, :])

        for l in range(L):
            # xa^T (feat-major) from node-major x_sb
            for c in range(CH):
                pt = psA.tile([128, 128], DT, tag="tp")
                nc.tensor.transpose(out=pt[:], in_=x_sb[:, c, :], identity=ident[:])
                nc.vector.tensor_copy(out=xaT[:, c * 128:(c + 1) * 128], in_=pt[:])
            # W_l^T tile: W_d[l] is [out,in]; need [in,out]
            wl = strp.tile([128, DIM], DT, tag="wl")
            nc.sync.dma_start(wl[:], W_d[l])
            wpt = psA.tile([128, 128], DT, tag="tp")
            nc.tensor.transpose(out=wpt[:], in_=wl[:], identity=ident[:])
            nc.vector.tensor_copy(out=wt_sb[:], in_=wpt[:])
            if l > 0:
                # BN affine in feat-major: xaT = xaT*s + t (s,t per-partition)
                nc.vector.scalar_tensor_tensor(
                    out=xaT[:], in0=xaT[:], scalar=s_col[:, 0:1],
                    in1=t_col[:, 0:1].to_broadcast([128, SP]),
                    op0=AL.mult, op1=AL.add)
            # y shard
            for c in range(CH):
                yp = psA.tile([128, 128], DT, tag="yp")
                nc.tensor.matmul(out=yp[:], lhsT=xaT[:, c * 128:(c + 1) * 128],
                                 rhs=wt_sb[:], start=True, stop=True)
                ych = strp.tile([128, 128], DT, tag="ych")
                nc.vector.tensor_scalar_mul(ych[:], yp[:], dinv_sb[:, c:c + 1])
                nc.sync.dma_start(y_loc[c * 128:(c + 1) * 128, :], ych[:])
            brl = strp.tile([1, DIM], DT, tag="brl")
            nc.sync.dma_start(brl[:], b_d[l:l + 1, :])
            nc.gpsimd.partition_broadcast(b_bc[:], brl[:])
            y_all = y_alls[l]
            nc.gpsimd.collective_compute(
                "AllGather", AL.bypass,
                replica_groups=[list(range(NC))],
                ins=[y_loc[:].opt()],
                outs=[y_all[:].opt()])
            # aggregation
            for c in range(CH):
                R = Rc[c]
                o0 = Roff[c]
                acc = accp.tile([128, max(Rc), 128], DT, tag="acc")
                for k in range(R):
                    nc.gpsimd.indirect_dma_start(
                        out=acc[:, k, :], out_offset=None,
                        in_=y_all[:],
                        in_offset=bass.IndirectOffsetOnAxis(
                            ap=gidx_sb[:, o0 + k:o0 + k + 1], axis=0))
                # red = self term (own y rows, ew=1) + sqrt(deg)*b (bias edge)
                red = strp.tile([128, 128], DT, tag="red")
                nc.sync.dma_start(red[:], y_loc[c * 128:(c + 1) * 128, :])
                nc.vector.scalar_tensor_tensor(
                    out=red[:], in0=b_bc[:], scalar=sqd_sb[:, c:c + 1], in1=red[:],
                    op0=AL.mult, op1=AL.add)
                # fused per-round scale+accumulate: red = acc_k * ew_k + red
                for k in range(R):
                    nc.vector.scalar_tensor_tensor(
                        out=red[:], in0=acc[:, k, :], scalar=gew_sb[:, o0 + k:o0 + k + 1],
                        in1=red[:], op0=AL.mult, op1=AL.add)
                nc.scalar.activation(out=x_sb[:, c, :], in_=red[:],
                                     func=ACTF.Relu, scale=dinv_sb[:, c:c + 1])
                nc.sync.dma_start(xout[l][c * 128:(c + 1) * 128, :], x_sb[:, c, :])
            # stats
            s1p = psB.tile([128, 1], DT, tag="s1")
            s2p = psB.tile([128, 1], DT, tag="s2")
            for c in range(CH):
                xsq = strp.tile([128, 128], DT, tag="xsq")
                nc.vector.tensor_tensor(out=xsq[:], in0=x_sb[:, c, :], in1=x_sb[:, c, :], op=AL.mult)
                nc.tensor.matmul(out=s1p[:], lhsT=x_sb[:, c, :], rhs=mask_sb[:, c:c + 1],
                                 start=(c == 0), stop=(c == CH - 1))
                nc.tensor.matmul(out=s2p[:], lhsT=xsq[:], rhs=mask_sb[:, c:c + 1],
                                 start=(c == 0), stop=(c == CH - 1))
            stt = strp.tile([DIM, 2], DT, tag="stt")
            nc.vector.tensor_copy(out=stt[:, 0:1], in_=s1p[:])
            nc.vector.tensor_copy(out=stt[:, 1:2], in_=s2p[:])
            nc.sync.dma_start(st_loc[:], stt[:])
            st_all = st_alls[l]
            nc.gpsimd.collective_compute(
                "AllReduce", AL.add,
                replica_groups=[list(range(NC))],
                ins=[st_loc[:].opt()], outs=[st_all[:].opt()])
            sta = strp.tile([DIM, 2], DT, tag="sta")
            nc.sync.dma_start(sta[:], st_all[:])
            nc.sync.dma_start(stats_out[l], sta[:])
            # pooling of raw x
            pl = psB.tile([GW, 128], DT, tag="pl")
            for c in range(CH):
                plsb = strp.tile([128, GW], DT, tag="plsb")
                nc.sync.dma_start(plsb[:], pool_d[c * 128:(c + 1) * 128, :])
                nc.tensor.matmul(out=pl[:], lhsT=plsb[:], rhs=x_sb[:, c, :],
                                 start=(c == 0), stop=(c == CH - 1))
            plo = strp.tile([GW, 128], DT, tag="plo")
            nc.vector.tensor_copy(out=plo[:], in_=pl[:])
            nc.sync.dma_start(pool_out[l], plo[:])
            # BN affine for next layer, applied in-place to x_sb
            if l < L - 1:
                mu = smallp[:, 0:1]
                var = smallp[:, 1:2]
                inv = smallp[:, 2:3]
                musq = smallp[:, 3:4]
                sq = smallp[:, 4:5]
                ms = smallp[:, 5:6]
                nc.vector.tensor_scalar_mul(mu, sta[:, 0:1], 1.0 / float(N))
                nc.vector.tensor_scalar_mul(var, sta[:, 1:2], 1.0 / float(N))
                nc.vector.tensor_tensor(out=musq, in0=mu, in1=mu, op=AL.mult)
                nc.vector.tensor_tensor(out=var, in0=var, in1=musq, op=AL.subtract)
                nc.vector.tensor_scalar_add(var, var, BN_EPS)
                nc.scalar.activation(out=sq, in_=var, func=ACTF.Sqrt)
                nc.vector.reciprocal(out=inv, in_=sq)
                gcol = strp.tile([128, 1], DT, tag="gcol")
                bcol = strp.tile([128, 1], DT, tag="bcol")
                nc.sync.dma_start(gcol[:], gam_d[:, l:l + 1])
                nc.sync.dma_start(bcol[:], bet_d[:, l:l + 1])
                nc.vector.tensor_tensor(out=s_col[:], in0=gcol[:], in1=inv, op=AL.mult)
                nc.vector.tensor_tensor(out=ms, in0=mu, in1=s_col[:], op=AL.mult)
                nc.vector.tensor_tensor(out=t_col[:], in0=bcol[:], in1=ms, op=AL.subtract)
    nc.compile()
    return nc


_CACHE = {}
LAST_EXEC_NS = None


def kernel(x, edge_index, edge_weight, batch, fc_w, W, b, gamma, beta):
    x = np.asarray(x, np.float32)
    W = np.asarray(W, np.float32)
    fc_w = np.asarray(fc_w, np.float32)
    b = np.asarray(b, np.float32)
    gamma = np.asarray(gamma, np.float32)
    beta = np.asarray(beta, np.float32)
    edge_index = np.asarray(edge_index)
    edge_weight = np.asarray(edge_weight, np.float32)
    batch = np.asarray(batch)

    plan = _plan(edge_index, edge_weight, batch)
    key = (tuple(plan["Rc"]), plan["RT"])
    if key not in _CACHE:
        _CACHE[key] = _build(plan["Rc"], plan["Roff"], plan["RT"])
    nc = _CACHE[key]

    Wf = np.stack([W[0] @ fc_w, W[1], W[2]]).astype(np.float32)
    in_maps = []
    for k in range(NC):
        perm = plan["perms"][k]
        xs = np.zeros((SP, F), np.float32)
        xs[:S] = x[perm]
        in_maps.append({
            "x_in": xs,
            "gidx": np.ascontiguousarray(plan["gidx"][k]),
            "gew": np.ascontiguousarray(plan["gew"][k]),
            "dinv_sb": np.ascontiguousarray(plan["dinv_sb"][k]),
            "sqd_sb": np.ascontiguousarray(plan["sqd_sb"][k]),
            "mask_sb": np.ascontiguousarray(plan["mask_sb"][k]),
            "pool_p": np.ascontiguousarray(plan["pool_p"][k]),
            "Wf": Wf, "bv": b, "gam": np.ascontiguousarray(gamma.T), "bet": np.ascontiguousarray(beta.T),
        })

    from concourse.bass_utils import run_bass_kernel_spmd
    import os
    kw = {}
    if os.environ.get("GNN_TRACE"):
        try:
            import types as _t, antenv  # noqa
            from trn_agent_boot.trn_boot import _ntff_profile_via_ctypes
            _m = _t.ModuleType("antenv.axon_hooks")
            _m.get_axon_ntff_profile_hook = lambda: _ntff_profile_via_ctypes("/opt/axon/libaxon_pjrt.so")
            sys.modules.setdefault("antenv.axon_hooks", _m)
            kw = dict(trace=True, tmpdir=os.environ.get("GNN_TRACE_DIR") or None)
        except Exception:
            kw = {}
    res = run_bass_kernel_spmd(nc, in_maps, core_ids=list(range(NC)), **kw)
    global LAST_EXEC_NS
    LAST_EXEC_NS = res.exec_time_ns

    hcat = np.empty((N, L * DIM), np.float32)
    out = np.zeros((G, L * DIM), np.float32)
    st = res.results[0]["stats_out"]
    cnt = np.bincount(batch.astype(np.int64), minlength=G).astype(np.float32)
    for l in range(L):
        S1, S2 = st[l, :, 0], st[l, :, 1]
        mu = S1 / N
        var = S2 / N - mu * mu
        inv = 1.0 / np.sqrt(var + BN_EPS)
        s = gamma[l] * inv
        t = beta[l] - mu * s
        for k in range(NC):
            xr = res.results[k]["xout"][l][:S]
            hcat[plan["perms"][k], l * DIM:(l + 1) * DIM] = xr * s[None, :] + t[None, :]
            pr = res.results[k]["pool_out"][l]
            g0 = plan["g0s"][k]
            hi = min(G, g0 + GW)
            out[g0:hi, l * DIM:(l + 1) * DIM] += pr[:hi - g0] * s[None, :]
        out[:, l * DIM:(l + 1) * DIM] += cnt[:, None] * t[None, :]
    return out, hcat
